# revision 6
# baseline (speedup 1.0000x reference)
"""Plan E: token-only indirect train + ONE bf16 ap_gather for rule+ref.

vs Plan D: the rule table (4MB f32) also moves into SBUF, as bf16 (2MB) to
halve its DMA drain, concatenated with the ref table (0.5MB bf16) into one
[P, 10240] bf16 SBUF table. A single ap_gather (128 idx slots per group: 64
rule + 64 ref) extracts both components; a {0,1} select mask + 16-wide
reduce recovers per-position values. The gather train shrinks to 4 indirect
DMAs (token only). bf16 introduces <=2^-9 relative error on rule/ref probs,
~1e-3 on the final loss — the harness gate is 2e-2.

Masked positions read token prob 1.0 (sentinel) and rule/ref contribute 0
via the select mask, so ln(1)=0 drops out.
"""

import os
import sys

import numpy as np

for _p in ("/opt/trn_rl_repo", "/root/.axon_site/_ro/trn_rl_repo"):
    if os.path.isdir(_p) and _p not in sys.path:
        sys.path.insert(0, _p)

import ml_dtypes

L_A, B = 128, 32
V_RULE, V_TOK, V_REF = 2048, 32000, 512
EPS = 1e-07
N_CORES = 8
L_SH = L_A // N_CORES
NPOS = L_SH * B                  # 512 positions per core
P = 128
J = NPOS // P                    # 4 positions per partition
N_FLAT = NPOS * V_TOK            # token-only flat buffer
ZERO_IDX = N_FLAT
ONE_IDX = N_FLAT + 1
TBL_W = J * V_RULE + J * V_REF   # 10240 bf16 per partition
REF_BASE = J * V_RULE            # 8192

# meta (int32 [P, 265]):
#   cols 0:4     token flat offsets (col = j)
#   col  4       f32 bits of -1/B
#   cols 5:261   f32 bits of select mask [P, 256] (per gathered bf16 pair-half)
#   cols 261:265 int16 pairs: ap_gather PAIR indices [P, 8]
META_W = 265

_CACHE = {}


def _build():
    import concourse.bacc as bacc
    import concourse.bass as bass
    import concourse.mybir as mybir
    import concourse.tile as tile
    from concourse import library_config

    f32 = mybir.dt.float32
    bf16 = mybir.dt.bfloat16
    i16 = mybir.dt.int16
    i32 = mybir.dt.int32
    alu = mybir.AluOpType

    nc = bacc.Bacc(
        "TRN2",
        target_bir_lowering=False,
        debug=False,
        enable_asserts=False,
        num_devices=N_CORES,
    )

    meta_d = nc.dram_tensor("meta", [P, META_W], i32, kind="ExternalInput").ap()
    flat_d = nc.dram_tensor(
        "probs_flat", [N_FLAT + 2, 1], f32, kind="ExternalInput"
    ).ap()
    tbl_d = nc.dram_tensor("tbl_bf16", [P, TBL_W], bf16, kind="ExternalInput").ap()
    out_d = nc.dram_tensor("out", [1, 1], f32, kind="ExternalOutput").ap()

    with tile.TileContext(nc) as tc:
        with (
            tc.tile_pool(name="sb", bufs=1) as pool,
            tc.tile_pool(name="ps", bufs=1, space="PSUM") as psum,
        ):
            nc.gpsimd.load_library(library_config.ap_gather)

            meta = pool.tile([P, META_W], i32)
            nc.sync.dma_start(out=meta[:], in_=meta_d[:])
            tbl = pool.tile([P, TBL_W], bf16)
            nc.sync.dma_start(out=tbl[:], in_=tbl_d[:])
            negw = meta[:, 4:5].bitcast(f32)
            val01 = meta[:, 5:261].bitcast(f32)
            apgidx = meta[:, 261:265].bitcast(i16)

            # token element gathers (sentinel-encoded offsets)
            gv = pool.tile([P, J], f32)
            for col in range(J):
                nc.gpsimd.indirect_dma_start(
                    out=gv[:, col:col + 1],
                    out_offset=None,
                    in_=flat_d[:],
                    in_offset=bass.IndirectOffsetOnAxis(
                        ap=meta[:, col:col + 1], axis=0
                    ),
                )

            # rule+ref: one ap_gather (d=2: bf16 pairs) + select-mask
            # (which also picks the correct pair half) + 32-wide reduce
            aout = pool.tile([P, 256], bf16)
            nc.gpsimd.ap_gather(
                out_ap=aout[:], in_ap=tbl[:], idxs_ap=apgidx,
                channels=P, num_elems=TBL_W // 2, d=2, num_idxs=128,
            )
            am = pool.tile([P, 256], f32)
            nc.vector.tensor_mul(out=am[:], in0=aout[:], in1=val01)
            rr = pool.tile([P, 2 * J], f32)
            nc.vector.reduce_sum(
                out=rr[:],
                in_=am[:].rearrange("p (j i) -> p j i", i=32),
                axis=mybir.AxisListType.X,
            )

            s = pool.tile([P, J], f32)
            nc.vector.tensor_add(out=s[:], in0=gv[:], in1=rr[:, 0:J])
            nc.vector.tensor_add(out=s[:], in0=s[:], in1=rr[:, J:2 * J])
            nc.vector.tensor_scalar(
                out=s[:], in0=s[:], scalar1=EPS, scalar2=None, op0=alu.max
            )

            ln = pool.tile([P, J], f32)
            nc.scalar.activation(
                out=ln[:], in_=s[:], func=mybir.ActivationFunctionType.Ln
            )
            rs = pool.tile([P, 1], f32)
            nc.vector.reduce_sum(out=rs[:], in_=ln[:], axis=mybir.AxisListType.X)

            acc = psum.tile([1, 1], f32)
            nc.tensor.matmul(out=acc[:], lhsT=rs[:], rhs=negw, start=True, stop=True)
            res = pool.tile([1, 1], f32)
            nc.scalar.copy(out=res[:], in_=acc[:])
            nc.sync.dma_start(out=out_d[:], in_=res[:])

    nc.compile()
    return nc


def get_nc():
    if "nc" not in _CACHE:
        _CACHE["nc"] = _build()
    return _CACHE["nc"]


# position q -> (partition, slot): p = 16*(q//64) + q%16, j = (q%64)//16
_Q = np.arange(NPOS, dtype=np.int64)
_QP = 16 * (_Q // 64) + _Q % 16
_QJ = (_Q % 64) // 16
_QMAP = np.empty((P, J), np.int64)
_QMAP[_QP, _QJ] = _Q


def make_in_maps(rule_probs, token_probs, reference_probs, ground_truth_actions, mask):
    rule_probs = np.ascontiguousarray(np.asarray(rule_probs, dtype=np.float32))
    token_probs = np.ascontiguousarray(np.asarray(token_probs, dtype=np.float32))
    reference_probs = np.ascontiguousarray(np.asarray(reference_probs, dtype=np.float32))
    gt = np.asarray(ground_truth_actions, dtype=np.int32)
    mask = np.asarray(mask, dtype=np.int32)

    negw_bits = np.float32(-1.0 / B).view(np.int32)
    gi = np.arange(64, dtype=np.int64)
    p_idx = np.arange(P)[:, None]
    q_of = 64 * (p_idx // 16) + gi[None, :]          # [P, 64]
    slot_live = (gi[None, :] % 16) == (p_idx % 16)   # [P, 64]

    in_maps = []
    for i in range(N_CORES):
        lo, hi = i * L_SH, (i + 1) * L_SH
        gt_sh = gt[lo:hi].reshape(NPOS, 3).astype(np.int64)
        m_sh = mask[lo:hi].reshape(NPOS)
        rule_sh = rule_probs[lo:hi].reshape(NPOS, V_RULE)
        ref_sh = reference_probs[lo:hi].reshape(NPOS, V_REF)

        meta = np.zeros((P, META_W), np.int32)
        # token offsets with sentinel encoding
        offs = _Q * V_TOK + np.clip(gt_sh[:, 1], 0, None)
        offs[gt_sh[:, 1] < 0] = ZERO_IDX
        offs[m_sh == 0] = ONE_IDX
        meta[_QP, _QJ] = offs.astype(np.int32)
        meta[:, 4] = negw_bits

        # select mask [P, 256]: cols 2i+h, slot i (0:64 rule, 64:128 ref),
        # h = which half of the gathered bf16 pair holds the target element
        rule_ok = (gt_sh[:, 0] >= 0) & (m_sh == 1)
        ref_ok = (gt_sh[:, 2] >= 0) & (m_sh == 1)
        half_r = (np.clip(gt_sh[:, 0], 0, None) & 1)[q_of]  # [P, 64]
        half_f = (np.clip(gt_sh[:, 2], 0, None) & 1)[q_of]
        val01 = np.zeros((P, 256), np.float32)
        live_r = slot_live & rule_ok[q_of]
        live_f = slot_live & ref_ok[q_of]
        cols = np.arange(64) * 2
        for h in (0, 1):
            val01[:, cols + h] = live_r & (half_r == h)
            val01[:, 128 + cols + h] = live_f & (half_f == h)
        meta[:, 5:261] = val01.view(np.int32)

        # ap_gather PAIR indices [P, 8] int16
        apgidx = np.zeros((P, 8), np.int16)
        for g in range(8):
            qg = 64 * g + gi
            ivr = (
                (V_RULE * (gi // 16) + np.clip(gt_sh[qg, 0], 0, None)) >> 1
            ).astype(np.int16)
            ivf = (
                (REF_BASE + V_REF * (gi // 16) + np.clip(gt_sh[qg, 2], 0, None)) >> 1
            ).astype(np.int16)
            apgidx[16 * g + gi % 16, gi // 16] = ivr
            apgidx[16 * g + gi % 16, 4 + gi // 16] = ivf
        meta[:, 261:265] = apgidx.view(np.int32)

        probs_flat = np.concatenate(
            [token_probs[lo:hi].reshape(-1), np.array([0.0, 1.0], np.float32)]
        )
        tbl = np.concatenate(
            [
                rule_sh[_QMAP.reshape(-1)].reshape(P, J * V_RULE),
                ref_sh[_QMAP.reshape(-1)].reshape(P, J * V_REF),
            ],
            axis=1,
        ).astype(ml_dtypes.bfloat16)
        in_maps.append(
            {
                "meta": meta,
                "probs_flat": probs_flat.reshape(-1, 1),
                "tbl_bf16": tbl,
            }
        )
    return in_maps


def run(inputs, trace=False, trace_cores=None):
    from concourse.bass_utils import run_bass_kernel_spmd

    nc = get_nc()
    in_maps = make_in_maps(**inputs)
    res = run_bass_kernel_spmd(
        nc,
        in_maps,
        core_ids=list(range(N_CORES)),
        trace=trace,
        trace_cores=trace_cores,
    )
    total = np.float64(0.0)
    for r in res.results:
        total += np.float64(r["out"].reshape(())[()])
    return np.asarray(total, dtype=np.float32), res


def kernel(**inputs) -> np.ndarray:
    out, _ = run(inputs)
    return out


# revision 9
# speedup vs baseline: 1.0878x; 1.0878x over previous
"""Trainium2 Bass kernel for nn_Loss_20933670601009 (gathered-prob NLL loss).

The loss touches 3 elements per (l, b) position: one gathered prob from each
of the rule/token/reference tables. Instead of streaming ~566MB of prob
tensors, each core fetches exactly the values it needs:

  - rule + token values (8 x 128 = 1024 per core): indirect-DMA element
    gathers straight from HBM. HW consumes ONE offset per partition row per
    instruction, so 8 instructions of [P,1] cover 2 components x 4
    positions-per-partition. All index arithmetic, validity (gt == -1) and
    mask handling is precomputed on the host into the offsets via two
    sentinel elements appended to the flat buffer (flat[N]=0.0, flat[N+1]=1.0;
    masked positions read prob 1.0 so ln(1)=0 drops out). The offsets ride in
    a minimal [P,8] first DMA so the gather train starts as early as possible.
  - reference values: the whole per-core reference table is only 1MB, so it
    is DMA'd into SBUF (overlapping the gather train) in an ap_gather-ready,
    host-pre-permuted layout; ONE gpsimd ap_gather instruction (~0.4us)
    extracts the per-position elements. ap_gather applies each index slot to
    all 16 partitions of a group, so only slots whose partition matches the
    position's home row carry real values; a host-shipped {0,1} mask and a
    16-wide strided reduce select them. The ap_gather ucode library is loaded
    as the FIRST gpsimd instruction so its load overlaps the prologue +
    gather train (the indirect-DMA ucode is resident and unaffected).

Position mapping (per core, NPOS=512): position q lives at partition
p = 16*(q//64) + q%16, slot j = (q%64)//16, which makes one ap_gather
(64 idx slots per 16-partition group) cover all 512 positions.

prob + (prob<eps)*eps is replaced by max(prob, eps): identical unless
0 < prob < 1e-7, which for sums of uniform(0,1) draws is a ~1e-21 event.

Sharding: data-parallel over L_a (128 rows -> 16 rows x 8 cores, 512
positions per core). Per-core partials are summed on the host; the on-device
-1/32 weight reproduces mean-over-batch of per-sequence sums.
"""

import os
import sys

import numpy as np

for _p in ("/opt/trn_rl_repo", "/root/.axon_site/_ro/trn_rl_repo"):
    if os.path.isdir(_p) and _p not in sys.path:
        sys.path.insert(0, _p)

L_A, B = 128, 32
V_RULE, V_TOK, V_REF = 2048, 32000, 512
EPS = 1e-07
N_CORES = 8
L_SH = L_A // N_CORES            # 16 sequence rows per core
NPOS = L_SH * B                  # 512 positions per core
P = 128                          # SBUF partitions
J = NPOS // P                    # 4 positions per partition
N_FLAT = NPOS * (V_RULE + V_TOK)  # rule || token flat buffer
ZERO_IDX = N_FLAT                # sentinel: flat[N_FLAT] = 0.0
ONE_IDX = N_FLAT + 1             # sentinel: flat[N_FLAT+1] = 1.0

# aux (int32 [P, 67]):
#   col  0      f32 bits of -1/B (matmul weight)
#   cols 1:65   f32 bits of the ap_gather select mask [P, 64]
#   cols 65:67  int16 pairs: ap_gather indices [P, 4]
AUX_W = 67
REF_HALF = J * V_REF // 2        # 1024

_CACHE = {}


def _build():
    """Build + compile the per-core Bass module (same NEFF on all 8 cores)."""
    import concourse.bacc as bacc
    import concourse.bass as bass
    import concourse.mybir as mybir
    import concourse.tile as tile
    from concourse import library_config

    f32 = mybir.dt.float32
    i16 = mybir.dt.int16
    i32 = mybir.dt.int32
    alu = mybir.AluOpType

    nc = bacc.Bacc(
        "TRN2",
        target_bir_lowering=False,
        debug=False,
        enable_asserts=False,
        num_devices=N_CORES,
    )

    offs_d = nc.dram_tensor("offs", [P, 2 * J], i32, kind="ExternalInput").ap()
    aux_d = nc.dram_tensor("aux", [P, AUX_W], i32, kind="ExternalInput").ap()
    flat_d = nc.dram_tensor(
        "probs_flat", [N_FLAT + 2, 1], f32, kind="ExternalInput"
    ).ap()
    ref_d = nc.dram_tensor("ref_shuf", [P, J * V_REF], f32, kind="ExternalInput").ap()
    out_d = nc.dram_tensor("out", [1, 1], f32, kind="ExternalOutput").ap()

    with tile.TileContext(nc) as tc:
        with (
            tc.tile_pool(name="sb", bufs=1) as pool,
            tc.tile_pool(name="ps", bufs=1, space="PSUM") as psum,
        ):
            # ucode load; overlaps the prologue + gather train below
            nc.gpsimd.load_library(library_config.ap_gather)

            offs = pool.tile([P, 2 * J], i32)
            nc.sync.dma_start(out=offs[:], in_=offs_d[:])
            reftbl = pool.tile([P, J * V_REF], f32)
            nc.sync.dma_start(out=reftbl[:], in_=ref_d[:])
            aux = pool.tile([P, AUX_W], i32)
            nc.sync.dma_start(out=aux[:], in_=aux_d[:])
            negw = aux[:, 0:1].bitcast(f32)
            val01 = aux[:, 1:65].bitcast(f32)
            apgidx = aux[:, 65:67].bitcast(i16)

            # rule + token element gathers (sentinel-encoded offsets)
            gv = pool.tile([P, 2 * J], f32)
            for col in range(2 * J):
                nc.gpsimd.indirect_dma_start(
                    out=gv[:, col:col + 1],
                    out_offset=None,
                    in_=flat_d[:],
                    in_offset=bass.IndirectOffsetOnAxis(
                        ap=offs[:, col:col + 1], axis=0
                    ),
                )

            # reference values: one ap_gather + select-mask + 16-wide reduce
            aout = pool.tile([P, 64], f32)
            nc.gpsimd.ap_gather(
                out_ap=aout[:], in_ap=reftbl[:], idxs_ap=apgidx,
                channels=P, num_elems=J * V_REF, d=1, num_idxs=64,
            )
            am = pool.tile([P, 64], f32)
            nc.vector.tensor_mul(out=am[:], in0=aout[:], in1=val01)
            refs = pool.tile([P, J], f32)
            nc.vector.reduce_sum(
                out=refs[:],
                in_=am[:].rearrange("p (j i) -> p j i", i=16),
                axis=mybir.AxisListType.X,
            )

            # s[p, j] = rule + token + ref
            s = pool.tile([P, J], f32)
            nc.vector.reduce_sum(
                out=s[:],
                in_=gv[:].rearrange("p (j c) -> p j c", c=2),
                axis=mybir.AxisListType.X,
            )
            nc.vector.tensor_add(out=s[:], in0=s[:], in1=refs[:])
            # max(prob, eps) ~ prob + (prob < eps) * eps (see module docstring)
            nc.vector.tensor_scalar(
                out=s[:], in0=s[:], scalar1=EPS, scalar2=None, op0=alu.max
            )

            ln = pool.tile([P, J], f32)
            nc.scalar.activation(
                out=ln[:], in_=s[:], func=mybir.ActivationFunctionType.Ln
            )
            rs = pool.tile([P, 1], f32)
            nc.vector.reduce_sum(out=rs[:], in_=ln[:], axis=mybir.AxisListType.X)

            # partition reduction via PE; weight -1/B folds negation + mean
            acc = psum.tile([1, 1], f32)
            nc.tensor.matmul(out=acc[:], lhsT=rs[:], rhs=negw, start=True, stop=True)
            res = pool.tile([1, 1], f32)
            nc.scalar.copy(out=res[:], in_=acc[:])
            nc.sync.dma_start(out=out_d[:], in_=res[:])

    nc.compile()
    return nc


def get_nc():
    if "nc" not in _CACHE:
        _CACHE["nc"] = _build()
    return _CACHE["nc"]


# position q -> (partition, slot): p = 16*(q//64) + q%16, j = (q%64)//16
_Q = np.arange(NPOS, dtype=np.int64)
_QP = 16 * (_Q // 64) + _Q % 16
_QJ = (_Q % 64) // 16
# qmap[p, j] = q
_QMAP = np.empty((P, J), np.int64)
_QMAP[_QP, _QJ] = _Q


def make_in_maps(rule_probs, token_probs, reference_probs, ground_truth_actions, mask):
    """Shard the full inputs into 8 per-core input maps."""
    rule_probs = np.ascontiguousarray(np.asarray(rule_probs, dtype=np.float32))
    token_probs = np.ascontiguousarray(np.asarray(token_probs, dtype=np.float32))
    reference_probs = np.ascontiguousarray(np.asarray(reference_probs, dtype=np.float32))
    gt = np.asarray(ground_truth_actions, dtype=np.int32)
    mask = np.asarray(mask, dtype=np.int32)

    negw_bits = np.float32(-1.0 / B).view(np.int32)
    gi = np.arange(64, dtype=np.int64)
    p_idx = np.arange(P)[:, None]
    q_of = 64 * (p_idx // 16) + gi[None, :]          # [P, 64]
    slot_live = (gi[None, :] % 16) == (p_idx % 16)   # [P, 64]

    in_maps = []
    for i in range(N_CORES):
        lo, hi = i * L_SH, (i + 1) * L_SH
        gt_sh = gt[lo:hi].reshape(NPOS, 3).astype(np.int64)
        m_sh = mask[lo:hi].reshape(NPOS)
        ref_sh = reference_probs[lo:hi].reshape(NPOS, V_REF)

        # rule/token offsets with sentinel encoding
        offs_t = np.zeros((P, 2 * J), np.int32)
        segs = (0, NPOS * V_RULE)
        vs = (V_RULE, V_TOK)
        for c in range(2):
            offs = segs[c] + _Q * vs[c] + np.clip(gt_sh[:, c], 0, None)
            offs[gt_sh[:, c] < 0] = ZERO_IDX
            offs[m_sh == 0] = ONE_IDX if c == 0 else ZERO_IDX
            offs_t[_QP, 2 * _QJ + c] = offs.astype(np.int32)

        aux = np.zeros((P, AUX_W), np.int32)
        aux[:, 0] = negw_bits
        # ap_gather select mask: slot i live on partition p iff i%16 == p%16
        # and the ref component of q = 64*(p//16)+i is valid & unmasked
        ref_ok = (gt_sh[:, 2] >= 0) & (m_sh == 1)
        val01 = np.zeros((P, 64), np.float32)
        val01[:] = slot_live & ref_ok[q_of]
        aux[:, 1:65] = val01.view(np.int32)
        # ap_gather indices: per group g, slot i -> 512*(i//16) + ref idx
        apgidx = np.zeros((P, 4), np.int16)
        for g in range(8):
            qg = 64 * g + gi
            iv = (V_REF * (gi // 16) + np.clip(gt_sh[qg, 2], 0, None)).astype(np.int16)
            apgidx[16 * g + gi % 16, gi // 16] = iv
        aux[:, 65:67] = apgidx.view(np.int32)

        probs_flat = np.concatenate(
            [
                rule_probs[lo:hi].reshape(-1),
                token_probs[lo:hi].reshape(-1),
                np.array([0.0, 1.0], np.float32),
            ]
        )
        ref_shuf = ref_sh[_QMAP.reshape(-1)].reshape(P, J * V_REF)
        in_maps.append(
            {
                "offs": offs_t,
                "aux": aux,
                "probs_flat": probs_flat.reshape(-1, 1),
                "ref_shuf": ref_shuf,
            }
        )
    return in_maps


def run(inputs, trace=False, trace_cores=None):
    """Run on the 8 NeuronCores; returns (scalar ndarray, BassKernelResults)."""
    from concourse.bass_utils import run_bass_kernel_spmd

    nc = get_nc()
    in_maps = make_in_maps(**inputs)
    res = run_bass_kernel_spmd(
        nc,
        in_maps,
        core_ids=list(range(N_CORES)),
        trace=trace,
        trace_cores=trace_cores,
    )
    total = np.float64(0.0)
    for r in res.results:
        total += np.float64(r["out"].reshape(())[()])
    return np.asarray(total, dtype=np.float32), res


def kernel(**inputs) -> np.ndarray:
    out, _ = run(inputs)
    return out


# revision 11
# speedup vs baseline: 1.1159x; 1.0258x over previous
"""Trainium2 Bass kernel for nn_Loss_20933670601009 (gathered-prob NLL loss).

The loss touches 3 elements per (l, b) position: one gathered prob from each
of the rule/token/reference tables. Instead of streaming ~566MB of prob
tensors, each core fetches exactly the values it needs:

  - rule + token values (8 x 128 = 1024 per core): indirect-DMA element
    gathers straight from HBM. HW consumes ONE offset per partition row per
    instruction, so 8 instructions of [P,1] cover 2 components x 4
    positions-per-partition. All index arithmetic, validity (gt == -1) and
    mask handling is precomputed on the host into the offsets via two
    sentinel elements appended to the flat buffer (flat[N]=0.0, flat[N+1]=1.0;
    masked positions read prob 1.0 so ln(1)=0 drops out). The offsets ride in
    a minimal [P,8] first DMA so the gather train starts as early as possible.
  - reference values: the whole per-core reference table is only 1MB, so it
    is DMA'd into SBUF (via SWDGE so descriptor generation is ~1us instead of
    HWDGE's ~14us issue, the drain overlapping the gather train) in an
    ap_gather-ready,
    host-pre-permuted layout; ONE gpsimd ap_gather instruction (~0.4us)
    extracts the per-position elements. ap_gather applies each index slot to
    all 16 partitions of a group, so only slots whose partition matches the
    position's home row carry real values; a host-shipped {0,1} mask and a
    16-wide strided reduce select them. The ap_gather ucode library is loaded
    as the FIRST gpsimd instruction so its load overlaps the prologue +
    gather train (the indirect-DMA ucode is resident and unaffected).

Position mapping (per core, NPOS=512): position q lives at partition
p = 16*(q//64) + q%16, slot j = (q%64)//16, which makes one ap_gather
(64 idx slots per 16-partition group) cover all 512 positions.

prob + (prob<eps)*eps is replaced by max(prob, eps): identical unless
0 < prob < 1e-7, which for sums of uniform(0,1) draws is a ~1e-21 event.

Sharding: data-parallel over L_a (128 rows -> 16 rows x 8 cores, 512
positions per core). Per-core partials are summed on the host; the on-device
-1/32 weight reproduces mean-over-batch of per-sequence sums.
"""

import os
import sys

import numpy as np

for _p in ("/opt/trn_rl_repo", "/root/.axon_site/_ro/trn_rl_repo"):
    if os.path.isdir(_p) and _p not in sys.path:
        sys.path.insert(0, _p)

L_A, B = 128, 32
V_RULE, V_TOK, V_REF = 2048, 32000, 512
EPS = 1e-07
N_CORES = 8
L_SH = L_A // N_CORES            # 16 sequence rows per core
NPOS = L_SH * B                  # 512 positions per core
P = 128                          # SBUF partitions
J = NPOS // P                    # 4 positions per partition
N_FLAT = NPOS * (V_RULE + V_TOK)  # rule || token flat buffer
ZERO_IDX = N_FLAT                # sentinel: flat[N_FLAT] = 0.0
ONE_IDX = N_FLAT + 1             # sentinel: flat[N_FLAT+1] = 1.0

# aux (int32 [P, 67]):
#   col  0      f32 bits of -1/B (matmul weight)
#   cols 1:65   f32 bits of the ap_gather select mask [P, 64]
#   cols 65:67  int16 pairs: ap_gather indices [P, 4]
AUX_W = 67
REF_HALF = J * V_REF // 2        # 1024

_CACHE = {}


def _build():
    """Build + compile the per-core Bass module (same NEFF on all 8 cores)."""
    import concourse.bacc as bacc
    import concourse.bass as bass
    import concourse.mybir as mybir
    import concourse.tile as tile
    from concourse import library_config

    f32 = mybir.dt.float32
    i16 = mybir.dt.int16
    i32 = mybir.dt.int32
    alu = mybir.AluOpType

    nc = bacc.Bacc(
        "TRN2",
        target_bir_lowering=False,
        debug=False,
        enable_asserts=False,
        num_devices=N_CORES,
    )

    offs_d = nc.dram_tensor("offs", [P, 2 * J], i32, kind="ExternalInput").ap()
    aux_d = nc.dram_tensor("aux", [P, AUX_W], i32, kind="ExternalInput").ap()
    flat_d = nc.dram_tensor(
        "probs_flat", [N_FLAT + 2, 1], f32, kind="ExternalInput"
    ).ap()
    ref_d = nc.dram_tensor("ref_shuf", [P, J * V_REF], f32, kind="ExternalInput").ap()
    out_d = nc.dram_tensor("out", [1, 1], f32, kind="ExternalOutput").ap()

    with tile.TileContext(nc) as tc:
        with (
            tc.tile_pool(name="sb", bufs=1) as pool,
            tc.tile_pool(name="ps", bufs=1, space="PSUM") as psum,
        ):
            # ucode load; overlaps the prologue + gather train below
            nc.gpsimd.load_library(library_config.ap_gather)

            # ref table via SWDGE: HWDGE issues descriptors at ~110ns each
            # (128 partition rows -> ~14us drain), SWDGE generates them in
            # ~1us and the drain overlaps the gather train below
            reftbl = pool.tile([P, J * V_REF], f32)
            nc.gpsimd.dma_start(out=reftbl[:], in_=ref_d[:])

            offs = pool.tile([P, 2 * J], i32)
            nc.sync.dma_start(out=offs[:], in_=offs_d[:])
            aux = pool.tile([P, AUX_W], i32)
            nc.sync.dma_start(out=aux[:], in_=aux_d[:])
            negw = aux[:, 0:1].bitcast(f32)
            val01 = aux[:, 1:65].bitcast(f32)
            apgidx = aux[:, 65:67].bitcast(i16)

            # rule + token element gathers (sentinel-encoded offsets)
            gv = pool.tile([P, 2 * J], f32)
            for col in range(2 * J):
                nc.gpsimd.indirect_dma_start(
                    out=gv[:, col:col + 1],
                    out_offset=None,
                    in_=flat_d[:],
                    in_offset=bass.IndirectOffsetOnAxis(
                        ap=offs[:, col:col + 1], axis=0
                    ),
                )

            # reference values: one ap_gather + select-mask + 16-wide reduce
            aout = pool.tile([P, 64], f32)
            nc.gpsimd.ap_gather(
                out_ap=aout[:], in_ap=reftbl[:], idxs_ap=apgidx,
                channels=P, num_elems=J * V_REF, d=1, num_idxs=64,
            )
            am = pool.tile([P, 64], f32)
            nc.vector.tensor_mul(out=am[:], in0=aout[:], in1=val01)
            refs = pool.tile([P, J], f32)
            nc.vector.reduce_sum(
                out=refs[:],
                in_=am[:].rearrange("p (j i) -> p j i", i=16),
                axis=mybir.AxisListType.X,
            )

            # s[p, j] = rule + token + ref
            s = pool.tile([P, J], f32)
            nc.vector.reduce_sum(
                out=s[:],
                in_=gv[:].rearrange("p (j c) -> p j c", c=2),
                axis=mybir.AxisListType.X,
            )
            nc.vector.tensor_add(out=s[:], in0=s[:], in1=refs[:])
            # max(prob, eps) ~ prob + (prob < eps) * eps (see module docstring)
            nc.vector.tensor_scalar(
                out=s[:], in0=s[:], scalar1=EPS, scalar2=None, op0=alu.max
            )

            ln = pool.tile([P, J], f32)
            nc.scalar.activation(
                out=ln[:], in_=s[:], func=mybir.ActivationFunctionType.Ln
            )
            rs = pool.tile([P, 1], f32)
            nc.vector.reduce_sum(out=rs[:], in_=ln[:], axis=mybir.AxisListType.X)

            # partition reduction via PE; weight -1/B folds negation + mean
            acc = psum.tile([1, 1], f32)
            nc.tensor.matmul(out=acc[:], lhsT=rs[:], rhs=negw, start=True, stop=True)
            res = pool.tile([1, 1], f32)
            nc.scalar.copy(out=res[:], in_=acc[:])
            nc.sync.dma_start(out=out_d[:], in_=res[:])

    nc.compile()
    return nc


def get_nc():
    if "nc" not in _CACHE:
        _CACHE["nc"] = _build()
    return _CACHE["nc"]


# position q -> (partition, slot): p = 16*(q//64) + q%16, j = (q%64)//16
_Q = np.arange(NPOS, dtype=np.int64)
_QP = 16 * (_Q // 64) + _Q % 16
_QJ = (_Q % 64) // 16
# qmap[p, j] = q
_QMAP = np.empty((P, J), np.int64)
_QMAP[_QP, _QJ] = _Q


def make_in_maps(rule_probs, token_probs, reference_probs, ground_truth_actions, mask):
    """Shard the full inputs into 8 per-core input maps."""
    rule_probs = np.ascontiguousarray(np.asarray(rule_probs, dtype=np.float32))
    token_probs = np.ascontiguousarray(np.asarray(token_probs, dtype=np.float32))
    reference_probs = np.ascontiguousarray(np.asarray(reference_probs, dtype=np.float32))
    gt = np.asarray(ground_truth_actions, dtype=np.int32)
    mask = np.asarray(mask, dtype=np.int32)

    negw_bits = np.float32(-1.0 / B).view(np.int32)
    gi = np.arange(64, dtype=np.int64)
    p_idx = np.arange(P)[:, None]
    q_of = 64 * (p_idx // 16) + gi[None, :]          # [P, 64]
    slot_live = (gi[None, :] % 16) == (p_idx % 16)   # [P, 64]

    in_maps = []
    for i in range(N_CORES):
        lo, hi = i * L_SH, (i + 1) * L_SH
        gt_sh = gt[lo:hi].reshape(NPOS, 3).astype(np.int64)
        m_sh = mask[lo:hi].reshape(NPOS)
        ref_sh = reference_probs[lo:hi].reshape(NPOS, V_REF)

        # rule/token offsets with sentinel encoding
        offs_t = np.zeros((P, 2 * J), np.int32)
        segs = (0, NPOS * V_RULE)
        vs = (V_RULE, V_TOK)
        for c in range(2):
            offs = segs[c] + _Q * vs[c] + np.clip(gt_sh[:, c], 0, None)
            offs[gt_sh[:, c] < 0] = ZERO_IDX
            offs[m_sh == 0] = ONE_IDX if c == 0 else ZERO_IDX
            offs_t[_QP, 2 * _QJ + c] = offs.astype(np.int32)

        aux = np.zeros((P, AUX_W), np.int32)
        aux[:, 0] = negw_bits
        # ap_gather select mask: slot i live on partition p iff i%16 == p%16
        # and the ref component of q = 64*(p//16)+i is valid & unmasked
        ref_ok = (gt_sh[:, 2] >= 0) & (m_sh == 1)
        val01 = np.zeros((P, 64), np.float32)
        val01[:] = slot_live & ref_ok[q_of]
        aux[:, 1:65] = val01.view(np.int32)
        # ap_gather indices: per group g, slot i -> 512*(i//16) + ref idx
        apgidx = np.zeros((P, 4), np.int16)
        for g in range(8):
            qg = 64 * g + gi
            iv = (V_REF * (gi // 16) + np.clip(gt_sh[qg, 2], 0, None)).astype(np.int16)
            apgidx[16 * g + gi % 16, gi // 16] = iv
        aux[:, 65:67] = apgidx.view(np.int32)

        probs_flat = np.concatenate(
            [
                rule_probs[lo:hi].reshape(-1),
                token_probs[lo:hi].reshape(-1),
                np.array([0.0, 1.0], np.float32),
            ]
        )
        ref_shuf = ref_sh[_QMAP.reshape(-1)].reshape(P, J * V_REF)
        in_maps.append(
            {
                "offs": offs_t,
                "aux": aux,
                "probs_flat": probs_flat.reshape(-1, 1),
                "ref_shuf": ref_shuf,
            }
        )
    return in_maps


def run(inputs, trace=False, trace_cores=None):
    """Run on the 8 NeuronCores; returns (scalar ndarray, BassKernelResults)."""
    from concourse.bass_utils import run_bass_kernel_spmd

    nc = get_nc()
    in_maps = make_in_maps(**inputs)
    res = run_bass_kernel_spmd(
        nc,
        in_maps,
        core_ids=list(range(N_CORES)),
        trace=trace,
        trace_cores=trace_cores,
    )
    total = np.float64(0.0)
    for r in res.results:
        total += np.float64(r["out"].reshape(())[()])
    return np.asarray(total, dtype=np.float32), res


def kernel(**inputs) -> np.ndarray:
    out, _ = run(inputs)
    return out


# revision 13
# speedup vs baseline: 1.1485x; 1.0292x over previous
"""Trainium2 Bass kernel for nn_Loss_20933670601009 (gathered-prob NLL loss).

The loss touches 3 elements per (l, b) position: one gathered prob from each
of the rule/token/reference tables. Instead of streaming ~566MB of prob
tensors, each core fetches exactly the values it needs:

  - rule + token values (8 x 128 = 1024 per core): indirect-DMA element
    gathers straight from HBM. HW consumes ONE offset per partition row per
    instruction, so 8 instructions of [P,1] cover 2 components x 4
    positions-per-partition. All index arithmetic, validity (gt == -1) and
    mask handling is precomputed on the host into the offsets via two
    sentinel elements appended to the flat buffer (flat[N]=0.0, flat[N+1]=1.0;
    masked positions read prob 1.0 so ln(1)=0 drops out). The offsets ride in
    a minimal [P,8] first DMA so the gather train starts as early as possible.
  - reference values: the whole per-core reference table is only 1MB, so it
    is DMA'd into SBUF (overlapping the gather train) in an ap_gather-ready,
    host-pre-permuted layout; ONE gpsimd ap_gather instruction (~0.4us)
    extracts the per-position elements. ap_gather applies each index slot to
    all 16 partitions of a group, so only slots whose partition matches the
    position's home row carry real values; a host-shipped {0,1} mask and a
    16-wide strided reduce select them. The ap_gather ucode library is loaded
    as the FIRST gpsimd instruction so its load overlaps the prologue +
    gather train (the indirect-DMA ucode is resident and unaffected).

Position mapping (per core, NPOS=512): position q lives at partition
p = 16*(q//64) + q%16, slot j = (q%64)//16, which makes one ap_gather
(64 idx slots per 16-partition group) cover all 512 positions.

prob + (prob<eps)*eps is replaced by max(prob, eps): identical unless
0 < prob < 1e-7, which for sums of uniform(0,1) draws is a ~1e-21 event.

Sharding: data-parallel over L_a (128 rows -> 16 rows x 8 cores, 512
positions per core). Per-core partials are summed on the host; the on-device
-1/32 weight reproduces mean-over-batch of per-sequence sums.
"""

import os
import sys

import numpy as np

for _p in ("/opt/trn_rl_repo", "/root/.axon_site/_ro/trn_rl_repo"):
    if os.path.isdir(_p) and _p not in sys.path:
        sys.path.insert(0, _p)

L_A, B = 128, 32
V_RULE, V_TOK, V_REF = 2048, 32000, 512
EPS = 1e-07
N_CORES = 8
L_SH = L_A // N_CORES            # 16 sequence rows per core
NPOS = L_SH * B                  # 512 positions per core
P = 128                          # SBUF partitions
J = NPOS // P                    # 4 positions per partition
N_FLAT = NPOS * (V_RULE + V_TOK)  # rule || token flat buffer
ZERO_IDX = N_FLAT                # sentinel: flat[N_FLAT] = 0.0
ONE_IDX = N_FLAT + 1             # sentinel: flat[N_FLAT+1] = 1.0

# aux (int32 [P, 67]):
#   col  0      f32 bits of -1/B (matmul weight)
#   cols 1:65   f32 bits of the ap_gather select mask [P, 64]
#   cols 65:67  int16 pairs: ap_gather indices [P, 4]
AUX_W = 67
REF_HALF = J * V_REF // 2        # 1024

_CACHE = {}


def _build():
    """Build + compile the per-core Bass module (same NEFF on all 8 cores)."""
    import concourse.bacc as bacc
    import concourse.bass as bass
    import concourse.mybir as mybir
    import concourse.tile as tile
    from concourse import library_config

    f32 = mybir.dt.float32
    i16 = mybir.dt.int16
    i32 = mybir.dt.int32
    alu = mybir.AluOpType

    nc = bacc.Bacc(
        "TRN2",
        target_bir_lowering=False,
        debug=False,
        enable_asserts=False,
        num_devices=N_CORES,
    )

    offs_d = nc.dram_tensor("offs", [P, 2 * J], i32, kind="ExternalInput").ap()
    aux_d = nc.dram_tensor("aux", [P, AUX_W], i32, kind="ExternalInput").ap()
    flat_d = nc.dram_tensor(
        "probs_flat", [N_FLAT + 2, 1], f32, kind="ExternalInput"
    ).ap()
    ref_d = nc.dram_tensor("ref_shuf", [P, J * V_REF], f32, kind="ExternalInput").ap()
    out_d = nc.dram_tensor("out", [1, 1], f32, kind="ExternalOutput").ap()

    with tile.TileContext(nc) as tc:
        with (
            tc.tile_pool(name="sb", bufs=1) as pool,
            tc.tile_pool(name="ps", bufs=1, space="PSUM") as psum,
        ):
            # ucode load; overlaps the prologue + gather train below
            nc.gpsimd.load_library(library_config.ap_gather)

            offs = pool.tile([P, 2 * J], i32)
            nc.sync.dma_start(out=offs[:], in_=offs_d[:])
            reftbl = pool.tile([P, J * V_REF], f32)
            nc.sync.dma_start(out=reftbl[:], in_=ref_d[:])
            aux = pool.tile([P, AUX_W], i32)
            nc.sync.dma_start(out=aux[:], in_=aux_d[:])
            negw = aux[:, 0:1].bitcast(f32)
            val01 = aux[:, 1:65].bitcast(f32)
            apgidx = aux[:, 65:67].bitcast(i16)

            # rule + token element gathers (sentinel-encoded offsets)
            gv = pool.tile([P, 2 * J], f32)
            for col in range(2 * J):
                nc.gpsimd.indirect_dma_start(
                    out=gv[:, col:col + 1],
                    out_offset=None,
                    in_=flat_d[:],
                    in_offset=bass.IndirectOffsetOnAxis(
                        ap=offs[:, col:col + 1], axis=0
                    ),
                )

            # reference values: one ap_gather + select-mask + 16-wide reduce
            aout = pool.tile([P, 64], f32)
            nc.gpsimd.ap_gather(
                out_ap=aout[:], in_ap=reftbl[:], idxs_ap=apgidx,
                channels=P, num_elems=J * V_REF, d=1, num_idxs=64,
            )
            am = pool.tile([P, 64], f32)
            nc.vector.tensor_mul(out=am[:], in0=aout[:], in1=val01)
            refs = pool.tile([P, J], f32)
            nc.vector.reduce_sum(
                out=refs[:],
                in_=am[:].rearrange("p (j i) -> p j i", i=16),
                axis=mybir.AxisListType.X,
            )

            # s[p, j] = rule + token + ref
            s = pool.tile([P, J], f32)
            nc.vector.reduce_sum(
                out=s[:],
                in_=gv[:].rearrange("p (j c) -> p j c", c=2),
                axis=mybir.AxisListType.X,
            )
            nc.vector.tensor_add(out=s[:], in0=s[:], in1=refs[:])
            # max(prob, eps) ~ prob + (prob < eps) * eps (see module docstring)
            nc.vector.tensor_scalar(
                out=s[:], in0=s[:], scalar1=EPS, scalar2=None, op0=alu.max
            )

            ln = pool.tile([P, J], f32)
            nc.scalar.activation(
                out=ln[:], in_=s[:], func=mybir.ActivationFunctionType.Ln
            )
            rs = pool.tile([P, 1], f32)
            nc.vector.reduce_sum(out=rs[:], in_=ln[:], axis=mybir.AxisListType.X)

            # partition reduction via PE; weight -1/B folds negation + mean
            acc = psum.tile([1, 1], f32)
            nc.tensor.matmul(out=acc[:], lhsT=rs[:], rhs=negw, start=True, stop=True)
            res = pool.tile([1, 1], f32)
            nc.scalar.copy(out=res[:], in_=acc[:])
            nc.sync.dma_start(out=out_d[:], in_=res[:])

    nc.compile()
    return nc


def get_nc():
    if "nc" not in _CACHE:
        _CACHE["nc"] = _build()
    return _CACHE["nc"]


# position q -> (partition, slot): p = 16*(q//64) + q%16, j = (q%64)//16
_Q = np.arange(NPOS, dtype=np.int64)
_QP = 16 * (_Q // 64) + _Q % 16
_QJ = (_Q % 64) // 16
# qmap[p, j] = q
_QMAP = np.empty((P, J), np.int64)
_QMAP[_QP, _QJ] = _Q


def make_in_maps(rule_probs, token_probs, reference_probs, ground_truth_actions, mask):
    """Shard the full inputs into 8 per-core input maps."""
    rule_probs = np.ascontiguousarray(np.asarray(rule_probs, dtype=np.float32))
    token_probs = np.ascontiguousarray(np.asarray(token_probs, dtype=np.float32))
    reference_probs = np.ascontiguousarray(np.asarray(reference_probs, dtype=np.float32))
    gt = np.asarray(ground_truth_actions, dtype=np.int32)
    mask = np.asarray(mask, dtype=np.int32)

    negw_bits = np.float32(-1.0 / B).view(np.int32)
    gi = np.arange(64, dtype=np.int64)
    p_idx = np.arange(P)[:, None]
    q_of = 64 * (p_idx // 16) + gi[None, :]          # [P, 64]
    slot_live = (gi[None, :] % 16) == (p_idx % 16)   # [P, 64]

    in_maps = []
    for i in range(N_CORES):
        lo, hi = i * L_SH, (i + 1) * L_SH
        gt_sh = gt[lo:hi].reshape(NPOS, 3).astype(np.int64)
        m_sh = mask[lo:hi].reshape(NPOS)
        ref_sh = reference_probs[lo:hi].reshape(NPOS, V_REF)

        # rule/token offsets with sentinel encoding
        offs_t = np.zeros((P, 2 * J), np.int32)
        segs = (0, NPOS * V_RULE)
        vs = (V_RULE, V_TOK)
        for c in range(2):
            offs = segs[c] + _Q * vs[c] + np.clip(gt_sh[:, c], 0, None)
            offs[gt_sh[:, c] < 0] = ZERO_IDX
            offs[m_sh == 0] = ONE_IDX if c == 0 else ZERO_IDX
            offs_t[_QP, 2 * _QJ + c] = offs.astype(np.int32)

        aux = np.zeros((P, AUX_W), np.int32)
        aux[:, 0] = negw_bits
        # ap_gather select mask: slot i live on partition p iff i%16 == p%16
        # and the ref component of q = 64*(p//16)+i is valid & unmasked
        ref_ok = (gt_sh[:, 2] >= 0) & (m_sh == 1)
        val01 = np.zeros((P, 64), np.float32)
        val01[:] = slot_live & ref_ok[q_of]
        aux[:, 1:65] = val01.view(np.int32)
        # ap_gather indices: per group g, slot i -> 512*(i//16) + ref idx
        apgidx = np.zeros((P, 4), np.int16)
        for g in range(8):
            qg = 64 * g + gi
            iv = (V_REF * (gi // 16) + np.clip(gt_sh[qg, 2], 0, None)).astype(np.int16)
            apgidx[16 * g + gi % 16, gi // 16] = iv
        aux[:, 65:67] = apgidx.view(np.int32)

        probs_flat = np.concatenate(
            [
                rule_probs[lo:hi].reshape(-1),
                token_probs[lo:hi].reshape(-1),
                np.array([0.0, 1.0], np.float32),
            ]
        )
        ref_shuf = ref_sh[_QMAP.reshape(-1)].reshape(P, J * V_REF)
        in_maps.append(
            {
                "offs": offs_t,
                "aux": aux,
                "probs_flat": probs_flat.reshape(-1, 1),
                "ref_shuf": ref_shuf,
            }
        )
    return in_maps


def run(inputs, trace=False, trace_cores=None):
    """Run on the 8 NeuronCores; returns (scalar ndarray, BassKernelResults)."""
    from concourse.bass_utils import run_bass_kernel_spmd

    nc = get_nc()
    in_maps = make_in_maps(**inputs)
    res = run_bass_kernel_spmd(
        nc,
        in_maps,
        core_ids=list(range(N_CORES)),
        trace=trace,
        trace_cores=trace_cores,
    )
    total = np.float64(0.0)
    for r in res.results:
        total += np.float64(r["out"].reshape(())[()])
    return np.asarray(total, dtype=np.float32), res


def kernel(**inputs) -> np.ndarray:
    out, _ = run(inputs)
    return out


# revision 17
# speedup vs baseline: 1.1631x; 1.0128x over previous
"""Trainium2 Bass kernel for nn_Loss_20933670601009 (gathered-prob NLL loss).

The loss touches 3 elements per (l, b) position: one gathered prob from each
of the rule/token/reference tables. Instead of streaming ~566MB of prob
tensors, each core fetches exactly the values it needs:

  - rule + token values (8 x 128 = 1024 per core): indirect-DMA element
    gathers straight from HBM. HW consumes ONE offset per partition row per
    instruction, so 8 instructions of [P,1] cover 2 components x 4
    positions-per-partition. All index arithmetic, validity (gt == -1) and
    mask handling is precomputed on the host into the offsets via two
    sentinel elements appended to the flat buffer (flat[N]=0.0, flat[N+1]=1.0;
    masked positions read prob 1.0 so ln(1)=0 drops out). The offsets ride in
    a minimal [P,8] first DMA so the gather train starts as early as possible.
  - reference values: the whole per-core reference table is only 1MB, so it
    is DMA'd into SBUF (overlapping the gather train) in an ap_gather-ready,
    host-pre-permuted layout; ONE gpsimd ap_gather instruction (~0.4us)
    extracts the per-position elements. ap_gather applies each index slot to
    all 16 partitions of a group, so only slots whose partition matches the
    position's home row carry real values; a host-shipped {0,1} mask and a
    16-wide strided reduce select them. The ap_gather ucode library is loaded
    as the FIRST gpsimd instruction so its load overlaps the prologue +
    gather train (the indirect-DMA ucode is resident and unaffected).

Position mapping (per core, NPOS=512): position q lives at partition
p = 16*(q//64) + q%16, slot j = (q%64)//16, which makes one ap_gather
(64 idx slots per 16-partition group) cover all 512 positions.

prob + (prob<eps)*eps is replaced by max(prob, eps): identical unless
0 < prob < 1e-7, which for sums of uniform(0,1) draws is a ~1e-21 event.

Sharding: data-parallel over L_a (128 rows -> 16 rows x 8 cores, 512
positions per core). Per-core partials are summed on the host; the on-device
-1/32 weight reproduces mean-over-batch of per-sequence sums.
"""

import os
import sys

import numpy as np

for _p in ("/opt/trn_rl_repo", "/root/.axon_site/_ro/trn_rl_repo"):
    if os.path.isdir(_p) and _p not in sys.path:
        sys.path.insert(0, _p)

L_A, B = 128, 32
V_RULE, V_TOK, V_REF = 2048, 32000, 512
EPS = 1e-07
N_CORES = 8
L_SH = L_A // N_CORES            # 16 sequence rows per core
NPOS = L_SH * B                  # 512 positions per core
P = 128                          # SBUF partitions
J = NPOS // P                    # 4 positions per partition
N_FLAT = NPOS * (V_RULE + V_TOK)  # rule || token flat buffer
ZERO_IDX = N_FLAT                # sentinel: flat[N_FLAT] = 0.0
ONE_IDX = N_FLAT + 1             # sentinel: flat[N_FLAT+1] = 1.0

# aux (int32 [P, 67]):
#   col  0      f32 bits of -1/B (matmul weight)
#   cols 1:65   f32 bits of the ap_gather select mask [P, 64]
#   cols 65:67  int16 pairs: ap_gather indices [P, 4]
AUX_W = 67

_CACHE = {}


def _build():
    """Build + compile the per-core Bass module (same NEFF on all 8 cores)."""
    import concourse.bacc as bacc
    import concourse.bass as bass
    import concourse.mybir as mybir
    import concourse.tile as tile
    from concourse import library_config

    f32 = mybir.dt.float32
    i16 = mybir.dt.int16
    i32 = mybir.dt.int32
    alu = mybir.AluOpType

    nc = bacc.Bacc(
        "TRN2",
        target_bir_lowering=False,
        debug=False,
        enable_asserts=False,
        num_devices=N_CORES,
    )

    offs_d = nc.dram_tensor("offs", [P, 2 * J], i32, kind="ExternalInput").ap()
    aux_d = nc.dram_tensor("aux", [P, AUX_W], i32, kind="ExternalInput").ap()
    flat_d = nc.dram_tensor(
        "probs_flat", [N_FLAT + 2, 1], f32, kind="ExternalInput"
    ).ap()
    ref_d = nc.dram_tensor("ref_shuf", [P, J * V_REF], f32, kind="ExternalInput").ap()
    out_d = nc.dram_tensor("out", [J, 1], f32, kind="ExternalOutput").ap()

    with tile.TileContext(nc) as tc:
        with (
            tc.tile_pool(name="sb", bufs=1) as pool,
            tc.tile_pool(name="ps", bufs=1, space="PSUM") as psum,
        ):
            # ucode load; overlaps the prologue + gather train below
            nc.gpsimd.load_library(library_config.ap_gather)

            offs = pool.tile([P, 2 * J], i32)
            nc.sync.dma_start(out=offs[:], in_=offs_d[:])
            reftbl = pool.tile([P, J * V_REF], f32)
            nc.sync.dma_start(out=reftbl[:], in_=ref_d[:])
            aux = pool.tile([P, AUX_W], i32)
            nc.sync.dma_start(out=aux[:], in_=aux_d[:])
            negw = aux[:, 0:1].bitcast(f32)
            val01 = aux[:, 1:65].bitcast(f32)
            apgidx = aux[:, 65:67].bitcast(i16)

            # rule + token element gathers (sentinel-encoded offsets)
            gv = pool.tile([P, 2 * J], f32)
            for col in range(2 * J):
                nc.gpsimd.indirect_dma_start(
                    out=gv[:, col:col + 1],
                    out_offset=None,
                    in_=flat_d[:],
                    in_offset=bass.IndirectOffsetOnAxis(
                        ap=offs[:, col:col + 1], axis=0
                    ),
                )

            # reference values: one ap_gather + select-mask + 16-wide reduce
            aout = pool.tile([P, 64], f32)
            nc.gpsimd.ap_gather(
                out_ap=aout[:], in_ap=reftbl[:], idxs_ap=apgidx,
                channels=P, num_elems=J * V_REF, d=1, num_idxs=64,
            )
            am = pool.tile([P, 64], f32)
            nc.vector.tensor_mul(out=am[:], in0=aout[:], in1=val01)
            refs = pool.tile([P, J], f32)
            nc.vector.reduce_sum(
                out=refs[:],
                in_=am[:].rearrange("p (j i) -> p j i", i=16),
                axis=mybir.AxisListType.X,
            )

            # s[p, j] = rule + token + ref
            s = pool.tile([P, J], f32)
            nc.vector.reduce_sum(
                out=s[:],
                in_=gv[:].rearrange("p (j c) -> p j c", c=2),
                axis=mybir.AxisListType.X,
            )
            nc.vector.tensor_add(out=s[:], in0=s[:], in1=refs[:])
            # max(prob, eps) ~ prob + (prob < eps) * eps (see module docstring)
            nc.vector.tensor_scalar(
                out=s[:], in0=s[:], scalar1=EPS, scalar2=None, op0=alu.max
            )

            ln = pool.tile([P, J], f32)
            nc.scalar.activation(
                out=ln[:], in_=s[:], func=mybir.ActivationFunctionType.Ln
            )

            # partition reduction via PE; weight -1/B folds negation + mean.
            # The matmul contracts partitions directly from ln [P, J], so the
            # free-axis reduce moves to the host (sums 4 values per core).
            acc = psum.tile([J, 1], f32)
            nc.tensor.matmul(out=acc[:], lhsT=ln[:], rhs=negw, start=True, stop=True)
            res = pool.tile([J, 1], f32)
            nc.scalar.copy(out=res[:], in_=acc[:])
            nc.sync.dma_start(out=out_d[:], in_=res[:])

    nc.compile()
    return nc


def get_nc():
    if "nc" not in _CACHE:
        _CACHE["nc"] = _build()
    return _CACHE["nc"]


# position q -> (partition, slot): p = 16*(q//64) + q%16, j = (q%64)//16
_Q = np.arange(NPOS, dtype=np.int64)
_QP = 16 * (_Q // 64) + _Q % 16
_QJ = (_Q % 64) // 16
# qmap[p, j] = q
_QMAP = np.empty((P, J), np.int64)
_QMAP[_QP, _QJ] = _Q


def make_in_maps(rule_probs, token_probs, reference_probs, ground_truth_actions, mask):
    """Shard the full inputs into 8 per-core input maps."""
    rule_probs = np.ascontiguousarray(np.asarray(rule_probs, dtype=np.float32))
    token_probs = np.ascontiguousarray(np.asarray(token_probs, dtype=np.float32))
    reference_probs = np.ascontiguousarray(np.asarray(reference_probs, dtype=np.float32))
    gt = np.asarray(ground_truth_actions, dtype=np.int32)
    mask = np.asarray(mask, dtype=np.int32)

    negw_bits = np.float32(-1.0 / B).view(np.int32)
    gi = np.arange(64, dtype=np.int64)
    p_idx = np.arange(P)[:, None]
    q_of = 64 * (p_idx // 16) + gi[None, :]          # [P, 64]
    slot_live = (gi[None, :] % 16) == (p_idx % 16)   # [P, 64]

    in_maps = []
    for i in range(N_CORES):
        lo, hi = i * L_SH, (i + 1) * L_SH
        gt_sh = gt[lo:hi].reshape(NPOS, 3).astype(np.int64)
        m_sh = mask[lo:hi].reshape(NPOS)
        ref_sh = reference_probs[lo:hi].reshape(NPOS, V_REF)

        # rule/token offsets with sentinel encoding
        offs_t = np.zeros((P, 2 * J), np.int32)
        segs = (0, NPOS * V_RULE)
        vs = (V_RULE, V_TOK)
        for c in range(2):
            offs = segs[c] + _Q * vs[c] + np.clip(gt_sh[:, c], 0, None)
            offs[gt_sh[:, c] < 0] = ZERO_IDX
            offs[m_sh == 0] = ONE_IDX if c == 0 else ZERO_IDX
            offs_t[_QP, 2 * _QJ + c] = offs.astype(np.int32)

        aux = np.zeros((P, AUX_W), np.int32)
        aux[:, 0] = negw_bits
        # ap_gather select mask: slot i live on partition p iff i%16 == p%16
        # and the ref component of q = 64*(p//16)+i is valid & unmasked
        ref_ok = (gt_sh[:, 2] >= 0) & (m_sh == 1)
        val01 = np.zeros((P, 64), np.float32)
        val01[:] = slot_live & ref_ok[q_of]
        aux[:, 1:65] = val01.view(np.int32)
        # ap_gather indices: per group g, slot i -> 512*(i//16) + ref idx
        apgidx = np.zeros((P, 4), np.int16)
        for g in range(8):
            qg = 64 * g + gi
            iv = (V_REF * (gi // 16) + np.clip(gt_sh[qg, 2], 0, None)).astype(np.int16)
            apgidx[16 * g + gi % 16, gi // 16] = iv
        aux[:, 65:67] = apgidx.view(np.int32)

        probs_flat = np.concatenate(
            [
                rule_probs[lo:hi].reshape(-1),
                token_probs[lo:hi].reshape(-1),
                np.array([0.0, 1.0], np.float32),
            ]
        )
        ref_shuf = ref_sh[_QMAP.reshape(-1)].reshape(P, J * V_REF)
        in_maps.append(
            {
                "offs": offs_t,
                "aux": aux,
                "probs_flat": probs_flat.reshape(-1, 1),
                "ref_shuf": ref_shuf,
            }
        )
    return in_maps


def run(inputs, trace=False, trace_cores=None):
    """Run on the 8 NeuronCores; returns (scalar ndarray, BassKernelResults)."""
    from concourse.bass_utils import run_bass_kernel_spmd

    nc = get_nc()
    in_maps = make_in_maps(**inputs)
    res = run_bass_kernel_spmd(
        nc,
        in_maps,
        core_ids=list(range(N_CORES)),
        trace=trace,
        trace_cores=trace_cores,
    )
    total = np.float64(0.0)
    for r in res.results:
        total += np.float64(r["out"].sum(dtype=np.float64))
    return np.asarray(total, dtype=np.float32), res


def kernel(**inputs) -> np.ndarray:
    out, _ = run(inputs)
    return out
